# revision 24
# baseline (speedup 1.0000x reference)
"""Trainium2 Bass kernel for weighted-CE + structural-penalty loss.

Full inputs -> data-parallel shard over batch across 8 NeuronCores ->
per-core Bass kernel computes small fp32 partial sums -> host combines
in float64.

CE trick: the CE term is a plain sum over positions, so it is invariant
under any position permutation. The host sorts each core's positions by
target class into 8 fixed-size bands (PADLEN each, zero-padded), and
rotates the class axis within each band so the target class lands in
class-slot 0. On device the "gather" of the target logit is then just
the contiguous class-0 slice, and the per-position CE weight wt =
ce_weights[targets] (fp16, host-computed) folds into one elementwise
multiply. Padded positions have x=0, wt=0 and contribute exactly 0.

Device per core (positions F=2096 per partition, chunks [48,512x4]):
 - exp on ACT (fp16), se = sum_c exp via identity-lhsT matmuls
   accumulating in PSUM (PE), lse = Ln(se) on ACT.
 - g = sum wt*x_target, wl = sum wt*lse: one TT multiply each + ones
   matmul column reductions into PSUM.
 - penalty (original order, half-rows on 128 partitions): negated
   cumsum scan (rp-lp), row max (=-min P), pair terms via shifted TT
   products reduced by ones-matmuls; host chains the row halves and
   adds the one clamped boundary term.
"""

import numpy as np

import concourse.bass as bass
import concourse.mybir as mybir
import concourse.tile as tile
from concourse import bacc
from concourse.bass_utils import run_bass_kernel_spmd

B, S, C = 512, 4096, 8
PENALTY_WEIGHT = 0.1
NCORES = 8
RB = B // NCORES          # batch rows per core
N = RB * S                # real positions per core (262144)
P = 128                   # SBUF partitions
F = 2096                  # padded positions per partition
PADLEN = F * P // C       # positions per class band (33536)
NPAD = F * P              # padded positions per core
CHUNKS = [512, 512, 512, 512, 48]   # position chunks (sum = F)
SH = S // 2               # penalty half-row length
HALO = 3
SW = SH + HALO

F32 = mybir.dt.float32
F16 = mybir.dt.float16
BF16 = mybir.dt.bfloat16
F8 = mybir.dt.float8e4
OP = mybir.AluOpType
AF = mybir.ActivationFunctionType

# [1, x] reduction regions inside the PSUM "red" tile (3 banks).
# bank0: g main [0:256] + g tail [256:304]
# bank1: wl main [512:768] + wl tail [768:816]
# bank2: pz [1024:1280]
RED_G, RED_GT, RED_WL, RED_WLT, RED_PZ = 0, 256, 512, 768, 1024
RED_N = 1536
WIN = 256                 # ones-matmul window width


def _patch_act_tables():
    """Prefer the single table set containing Exp+Ln+Copy so the kernel
    pays one ACT_TABLE_LOAD instead of alternating per chunk."""
    import concourse.hw_specs as hw_specs
    if getattr(hw_specs, "_loss_kernel_tables_patched", False):
        return
    orig = hw_specs.get_activation_tables

    def patched(arch):
        t = orig(arch)
        pref = "natural_log_exp_and_others"
        if pref not in t:
            return t
        return {k: (v if k == pref else set()) for k, v in t.items()}

    hw_specs.get_activation_tables = patched
    bacc.get_activation_tables = patched
    hw_specs._loss_kernel_tables_patched = True


def build_program(compile=True):
    _patch_act_tables()
    nc = bacc.Bacc("TRN2", target_bir_lowering=False, debug=False)

    x_d = nc.dram_tensor("x", [P, F * C], F16, kind="ExternalInput").ap()
    wt_d = nc.dram_tensor("wt", [P, F], F16, kind="ExternalInput").ap()
    s_d = nc.dram_tensor("s", [P, SW], F16, kind="ExternalInput").ap()
    id_d = nc.dram_tensor("ident", [P, 2 * P], F8, kind="ExternalInput").ap()
    red_d = nc.dram_tensor("red", [1, RED_N], F32, kind="ExternalOutput").ap()
    ps_d = nc.dram_tensor("pscan", [P, 2], F32, kind="ExternalOutput").ap()

    nch = len(CHUNKS)
    off = [int(x) for x in np.cumsum([0] + CHUNKS)]

    with tile.TileContext(nc) as tc:
        with (
            tc.tile_pool(name="xb", bufs=1) as xb,
            tc.tile_pool(name="eb", bufs=3) as eb,
            tc.tile_pool(name="stat", bufs=1) as stat,
            tc.tile_pool(name="pen", bufs=1) as pen,
            tc.tile_pool(name="psum", bufs=1, space="PSUM") as psum,
        ):
            s_t = pen.tile([P, SW], F16)
            ident = stat.tile([P, 2, P], F8)
            ones = stat.tile([P, 1], F16)
            nc.vector.memset(ones, 1.0)
            wt_sb = stat.tile([P, F], F16)
            x_ts = [xb.tile([P, C, cw], F16, name=f"xt{k}", tag=f"x{k}")
                    for k, cw in enumerate(CHUNKS)]
            # input DMA doorbells all issued up front, in need order
            nc.sync.dma_start(out=x_ts[0], in_=x_d[:, 0:CHUNKS[0] * C])
            nc.sync.dma_start(out=s_t, in_=s_d)
            nc.sync.dma_start(out=ident, in_=id_d)
            for k in range(1, len(CHUNKS)):
                nc.sync.dma_start(
                    out=x_ts[k], in_=x_d[:, off[k] * C:off[k + 1] * C])
            nc.sync.dma_start(out=wt_sb, in_=wt_d)

            lse = stat.tile([P, F], F16)
            gm = stat.tile([P, F], F16)
            wlm = stat.tile([P, F], F16)
            # PSUM: 5 full banks for se + 3 banks for the reductions
            se_ps = [psum.tile([P, 512], F32, name=f"se{k}")
                     for k in range(nch)]
            red = psum.tile([1, RED_N], F32, name="red")
            started = set()

            def acc_mm(key, out, lhsT, rhs, last):
                st = key not in started
                started.add(key)
                nc.tensor.matmul(out, lhsT=lhsT, rhs=rhs, start=st, stop=last)

            # ---------------- penalty (starts immediately) ----
            lp_t = pen.tile([P, SH], F16)
            rp_t = pen.tile([P, SW], F16)
            e_t = pen.tile([P, SW - 1], F16)
            nc.vector.tensor_scalar(out=lp_t, in0=s_t[:, 0:SH], scalar1=1.0,
                                    scalar2=None, op0=OP.is_equal)
            nc.vector.tensor_scalar(out=rp_t, in0=s_t, scalar1=2.0,
                                    scalar2=None, op0=OP.is_equal)
            nc.vector.tensor_scalar(out=e_t, in0=s_t[:, 0:SW - 1], scalar1=3.0,
                                    scalar2=None, op0=OP.is_equal)

            # negated cumsum: p_t = running(rp - lp) = -P ; fp16 is exact
            # for integer values up to +-2048.
            p_t = pen.tile([P, SH], F16)
            nc.vector.tensor_tensor_scan(out=p_t, data0=rp_t[:, 0:SH],
                                         data1=lp_t, initial=0.0,
                                         op0=OP.add, op1=OP.subtract)
            pscan = stat.tile([P, 2], F32)
            nc.vector.tensor_copy(out=pscan[:, 0:1], in_=p_t[:, SH - 1:SH])
            nc.vector.tensor_reduce(out=pscan[:, 1:2], in_=p_t,
                                    axis=mybir.AxisListType.X, op=OP.max)
            nc.gpsimd.dma_start(out=ps_d, in_=pscan)

            # pair terms: pzv[j] = lp[j]*q[j],
            # q[j] = rp[j+1] + e[j+1]*(1.5*rp[j+2] + 2*e[j+2]*rp[j+3])
            w1a = pen.tile([P, SW - 2], F16)
            w1b = pen.tile([P, SW - 2], F16)
            w5a = pen.tile([P, SW - 2], F16)
            w5b = pen.tile([P, SW - 2], F16)
            qa = pen.tile([P, SH], F16)
            qb = pen.tile([P, SH], F16)
            pzv = pen.tile([P, SH], F16)
            # w1b[j] = 2*e[j+1]*rp[j+2]
            nc.vector.tensor_scalar(out=w1a, in0=s_t[:, 1:SW - 1], scalar1=3.0,
                                    scalar2=2.0, op0=OP.is_equal, op1=OP.mult)
            nc.vector.tensor_mul(w1b, w1a, rp_t[:, 2:SW])
            # w5b[j] = 1.5*rp[j+1] + w1b[j]
            nc.vector.tensor_scalar(out=w5a, in0=s_t[:, 1:SW - 1], scalar1=2.0,
                                    scalar2=1.5, op0=OP.is_equal, op1=OP.mult)
            nc.vector.tensor_add(w5b, w5a, w1b)
            # q[j] = e[j+1]*w5b[j+1] + rp[j+1]
            nc.vector.tensor_mul(qa, e_t[:, 1:SH + 1], w5b[:, 1:SH + 1])
            nc.vector.tensor_add(qb, qa, rp_t[:, 1:SH + 1])
            nc.vector.tensor_mul(pzv, lp_t, qb)
            for w in range(SH // WIN):
                acc_mm(("pz",), red[:, RED_PZ:RED_PZ + WIN],
                       lhsT=ones, rhs=pzv[:, w * WIN:(w + 1) * WIN],
                       last=(w == SH // WIN - 1))

            # ---------------- CE chunks ----------------
            for k, cw in enumerate(CHUNKS):
                x_t = x_ts[k]
                e_x = eb.tile([P, C, 512], F8, tag="e")
                nc.scalar.activation(e_x[:, :, 0:cw], x_t, AF.Exp)
                for cp in range(C // 2):
                    nc.tensor.matmul(se_ps[k][:, 0:cw], lhsT=ident,
                                     rhs=e_x[:, 2 * cp:2 * cp + 2, 0:cw],
                                     start=(cp == 0), stop=(cp == C // 2 - 1),
                                     perf_mode=mybir.MatmulPerfMode.DoubleRow)
                ksl = slice(off[k], off[k + 1])
                # target logit = class-slot 0 of the rotated layout
                nc.vector.tensor_mul(gm[:, ksl], wt_sb[:, ksl], x_t[:, 0, :])
                nc.scalar.activation(lse[:, ksl], se_ps[k][:, 0:cw], AF.Ln)
                nc.vector.tensor_mul(wlm[:, ksl], wt_sb[:, ksl], lse[:, ksl])
                # column reductions: 48-chunk into the tail regions,
                # 512-chunks in 256-wide windows into the main regions.
                if k == nch - 1:
                    acc_mm(("gt",), red[:, RED_GT:RED_GT + 48], lhsT=ones,
                           rhs=gm[:, off[k]:off[k] + 48], last=True)
                    acc_mm(("wlt",), red[:, RED_WLT:RED_WLT + 48], lhsT=ones,
                           rhs=wlm[:, off[k]:off[k] + 48], last=True)
                else:
                    for w in (2 * k, 2 * k + 1):
                        wsl = slice(w * WIN, (w + 1) * WIN)
                        acc_mm(("g",), red[:, RED_G:RED_G + WIN], lhsT=ones,
                               rhs=gm[:, wsl], last=(k == nch - 2 and w % 2 == 1))
                        acc_mm(("wl",), red[:, RED_WL:RED_WL + WIN], lhsT=ones,
                               rhs=wlm[:, wsl], last=(k == nch - 2 and w % 2 == 1))

            red_sb = stat.tile([1, RED_N], F32)
            nc.vector.tensor_copy(out=red_sb[:, 0:1280], in_=red[:, 0:1280])
            nc.scalar.activation(red_sb[:, 1280:RED_N], red[:, 1280:RED_N],
                                 AF.Copy)
            nc.sync.dma_start(out=red_d, in_=red_sb)

    if compile:
        nc.compile()
    return nc


_program = None


def _get_program():
    global _program
    if _program is None:
        _program = build_program()
    return _program


def _pair_boundary(s):
    """The only clamped boundary pair term not covered on device:
    4 * [s[S-3]==1][s[S-2]==3][s[S-1]==2] per row."""
    m = (s[:, -3] == 1) & (s[:, -2] == 3) & (s[:, -1] == 2)
    return 4.0 * float(m.sum())


def combine_partials(results, s_full, nnz):
    gs = 0.0
    wl = 0.0
    pz = 0.0
    pen = 0.0
    for r in results:
        red = r["red"].astype(np.float64).ravel()
        gs += red[RED_G:RED_G + WIN].sum() + red[RED_GT:RED_GT + 48].sum()
        wl += red[RED_WL:RED_WL + WIN].sum() + red[RED_WLT:RED_WLT + 48].sum()
        pz += red[RED_PZ:RED_PZ + WIN].sum()
        sc = r["pscan"].astype(np.float64)
        pf, mp = -sc[:, 0], -sc[:, 1]   # undo the negated scan
        pfa, mpa = pf[0:RB], mp[0:RB]
        pfb, mpb = pf[RB:P], mp[RB:P]
        pft = pfa + pfb
        mpt = np.minimum(mpa, pfa + mpb)
        pen += (pft - 2.0 * np.minimum(0.0, mpt)).sum()
    pen += 2.0 * pz
    pen += _pair_boundary(s_full)
    ce_loss = (wl - gs) / (B * S)
    penalty = pen / nnz
    return np.float32(ce_loss + PENALTY_WEIGHT * penalty)


def make_in_maps(logits, targets, predicted_structures, ce_weights):
    lg = np.asarray(logits, dtype=np.float16)
    t = np.asarray(targets, dtype=np.int64)
    w16 = np.asarray(ce_weights, dtype=np.float16)
    s = np.ascontiguousarray(
        np.asarray(predicted_structures).reshape(B, S), dtype=np.float16)
    import ml_dtypes
    eye = np.eye(P, dtype=ml_dtypes.float8_e4m3fn)
    ident = np.concatenate([eye, eye], axis=1)
    in_maps = []
    for core in range(NCORES):
        rows = slice(core * RB, (core + 1) * RB)
        tc = t[rows].ravel()
        cnt = np.bincount(tc, minlength=C)
        assert cnt.max() <= PADLEN, f"class count {cnt.max()} > PADLEN"
        perm = np.argsort(tc, kind="stable")
        xs = lg[rows].reshape(N, C)[perm]
        xp = np.zeros((NPAD, C), np.float16)
        wtp = np.zeros(NPAD, np.float16)
        pos = 0
        for c in range(C):
            band = xs[pos:pos + cnt[c]]
            # rotate class axis: target class -> slot 0
            xp[c * PADLEN:c * PADLEN + cnt[c]] = np.concatenate(
                [band[:, c:], band[:, :c]], axis=1)
            wtp[c * PADLEN:c * PADLEN + cnt[c]] = w16[c]
            pos += cnt[c]
        # [P, F, C] -> class-blocked per chunk [P, sum_k C*w]
        xp = xp.reshape(P, F, C)
        xcore = np.empty((P, F * C), np.float16)
        o = 0
        a = 0
        for cw in CHUNKS:
            blk = xp[:, a:a + cw, :].transpose(0, 2, 1)  # [P, C, cw]
            xcore[:, o:o + C * cw] = blk.reshape(P, C * cw)
            o += C * cw
            a += cw
        sc = s[rows]
        s_pack = np.zeros((P, SW), np.float16)
        s_pack[0:RB] = sc[:, 0:SW]
        s_pack[RB:P, 0:SH] = sc[:, SH:S]
        in_maps.append({
            "x": xcore,
            "wt": wtp.reshape(P, F),
            "s": s_pack,
            "ident": ident,
        })
    return in_maps


def kernel(logits, targets, predicted_structures, ce_weights):
    in_maps = make_in_maps(logits, targets, predicted_structures, ce_weights)
    t = np.asarray(targets)
    nnz = float(B * S - int((t == 0).sum()))
    s_full = np.asarray(predicted_structures).reshape(B, S)
    nc = _get_program()
    res = run_bass_kernel_spmd(nc, in_maps, core_ids=list(range(NCORES)))
    return combine_partials(res.results, s_full, nnz)


# revision 25
# speedup vs baseline: 1.1090x; 1.1090x over previous
"""Trainium2 Bass kernel for weighted-CE + structural-penalty loss.

Full inputs -> data-parallel shard over batch across 8 NeuronCores ->
per-core Bass kernel computes small fp32 partial sums -> host combines
in float64.

CE trick: the CE term is a plain sum over positions, so it is invariant
under any position permutation. The host sorts each core's positions by
target class into 8 fixed-size bands (PADLEN each, zero-padded), and
rotates the class axis within each band so the target class lands in
class-slot 0. On device the "gather" of the target logit is then just
the contiguous class-0 slice, and the per-position CE weight wt =
ce_weights[targets] (fp16, host-computed) folds into one elementwise
multiply. Padded positions have x=0, wt=0 and contribute exactly 0.

Device per core (positions F=2096 per partition, chunks [48,512x4]):
 - exp on ACT (fp16), se = sum_c exp via identity-lhsT matmuls
   accumulating in PSUM (PE), lse = Ln(se) on ACT.
 - g = sum wt*x_target, wl = sum wt*lse: one TT multiply each + ones
   matmul column reductions into PSUM.
 - penalty (original order, half-rows on 128 partitions): negated
   cumsum scan (rp-lp), row max (=-min P), pair terms via shifted TT
   products reduced by ones-matmuls; host chains the row halves and
   adds the one clamped boundary term.
"""

import numpy as np

import concourse.bass as bass
import concourse.mybir as mybir
import concourse.tile as tile
from concourse import bacc
from concourse.bass_utils import run_bass_kernel_spmd

B, S, C = 512, 4096, 8
PENALTY_WEIGHT = 0.1
NCORES = 8
RB = B // NCORES          # batch rows per core
N = RB * S                # real positions per core (262144)
P = 128                   # SBUF partitions
F = 2096                  # padded positions per partition
PADLEN = F * P // C       # positions per class band (33536)
NPAD = F * P              # padded positions per core
CHUNKS = [128, 512, 512, 512, 432]  # position chunks (sum = F)
SH = S // 2               # penalty half-row length
HALO = 3
SW = SH + HALO

F32 = mybir.dt.float32
F16 = mybir.dt.float16
BF16 = mybir.dt.bfloat16
F8 = mybir.dt.float8e4
OP = mybir.AluOpType
AF = mybir.ActivationFunctionType

# [1, x] reduction regions inside the PSUM "red" tile (4 banks).
# bank0: g main, bank1: wl main, bank2: pz, bank3: tails
RED_G, RED_WL, RED_PZ, RED_GT, RED_WLT = 0, 512, 1024, 1536, 1584
RED_N = 1632
WIN = 512                 # ones-matmul window width


def _patch_act_tables():
    """Prefer the single table set containing Exp+Ln+Copy so the kernel
    pays one ACT_TABLE_LOAD instead of alternating per chunk."""
    import concourse.hw_specs as hw_specs
    if getattr(hw_specs, "_loss_kernel_tables_patched", False):
        return
    orig = hw_specs.get_activation_tables

    def patched(arch):
        t = orig(arch)
        pref = "natural_log_exp_and_others"
        if pref not in t:
            return t
        return {k: (v if k == pref else set()) for k, v in t.items()}

    hw_specs.get_activation_tables = patched
    bacc.get_activation_tables = patched
    hw_specs._loss_kernel_tables_patched = True


def build_program(compile=True):
    _patch_act_tables()
    nc = bacc.Bacc("TRN2", target_bir_lowering=False, debug=False)

    x_d = nc.dram_tensor("x", [P, F * C], F8, kind="ExternalInput").ap()
    wt_d = nc.dram_tensor("wt", [P, F], F16, kind="ExternalInput").ap()
    s_d = nc.dram_tensor("s", [P, SW], F16, kind="ExternalInput").ap()
    id_d = nc.dram_tensor("ident", [P, P], F16, kind="ExternalInput").ap()
    red_d = nc.dram_tensor("red", [1, RED_N], F32, kind="ExternalOutput").ap()
    ps_d = nc.dram_tensor("pscan", [P, 2], F32, kind="ExternalOutput").ap()

    nch = len(CHUNKS)
    off = [int(x) for x in np.cumsum([0] + CHUNKS)]

    with tile.TileContext(nc) as tc:
        with (
            tc.tile_pool(name="xb", bufs=1) as xb,
            tc.tile_pool(name="eb", bufs=3) as eb,
            tc.tile_pool(name="stat", bufs=1) as stat,
            tc.tile_pool(name="pen", bufs=1) as pen,
            tc.tile_pool(name="psum", bufs=1, space="PSUM") as psum,
        ):
            s_t = pen.tile([P, SW], F16)
            ident = stat.tile([P, P], F16)
            ones = stat.tile([P, 1], F16)
            nc.vector.memset(ones, 1.0)
            wt_sb = stat.tile([P, F], F16)
            x_ts = [xb.tile([P, C, cw], F8, name=f"xt{k}", tag=f"x{k}")
                    for k, cw in enumerate(CHUNKS)]
            # input DMA doorbells all issued up front, in need order
            nc.sync.dma_start(out=x_ts[0], in_=x_d[:, 0:CHUNKS[0] * C])
            nc.sync.dma_start(out=s_t, in_=s_d)
            nc.sync.dma_start(out=ident, in_=id_d)
            for k in range(1, len(CHUNKS)):
                nc.sync.dma_start(
                    out=x_ts[k], in_=x_d[:, off[k] * C:off[k + 1] * C])
            nc.sync.dma_start(out=wt_sb, in_=wt_d)

            lse = stat.tile([P, F], F16)
            gm = stat.tile([P, F], F16)
            wlm = stat.tile([P, F], F16)
            # PSUM: 4 full banks for se (reused) + 4 for the reductions
            se_ps = [psum.tile([P, 512], F32, name=f"se{k}")
                     for k in range(4)]
            red = psum.tile([1, RED_N], F32, name="red")
            started = set()

            def acc_mm(key, out, lhsT, rhs, last):
                st = key not in started
                started.add(key)
                nc.tensor.matmul(out, lhsT=lhsT, rhs=rhs, start=st, stop=last)

            # ---------------- penalty (starts immediately) ----
            lp_t = pen.tile([P, SH], F16)
            rp_t = pen.tile([P, SW], F16)
            e_t = pen.tile([P, SW - 1], F16)
            nc.vector.tensor_scalar(out=lp_t, in0=s_t[:, 0:SH], scalar1=1.0,
                                    scalar2=None, op0=OP.is_equal)
            nc.vector.tensor_scalar(out=rp_t, in0=s_t, scalar1=2.0,
                                    scalar2=None, op0=OP.is_equal)
            nc.vector.tensor_scalar(out=e_t, in0=s_t[:, 0:SW - 1], scalar1=3.0,
                                    scalar2=None, op0=OP.is_equal)

            # negated cumsum: p_t = running(rp - lp) = -P ; fp16 is exact
            # for integer values up to +-2048.
            p_t = pen.tile([P, SH], F16)
            nc.vector.tensor_tensor_scan(out=p_t, data0=rp_t[:, 0:SH],
                                         data1=lp_t, initial=0.0,
                                         op0=OP.add, op1=OP.subtract)
            pscan = stat.tile([P, 2], F32)
            nc.vector.tensor_copy(out=pscan[:, 0:1], in_=p_t[:, SH - 1:SH])
            nc.vector.tensor_reduce(out=pscan[:, 1:2], in_=p_t,
                                    axis=mybir.AxisListType.X, op=OP.max)
            nc.gpsimd.dma_start(out=ps_d, in_=pscan)

            # pair terms: pzv[j] = lp[j]*q[j],
            # q[j] = rp[j+1] + e[j+1]*(1.5*rp[j+2] + 2*e[j+2]*rp[j+3])
            w1a = pen.tile([P, SW - 2], F16)
            w1b = pen.tile([P, SW - 2], F16)
            w5a = pen.tile([P, SW - 2], F16)
            w5b = pen.tile([P, SW - 2], F16)
            qa = pen.tile([P, SH], F16)
            qb = pen.tile([P, SH], F16)
            pzv = pen.tile([P, SH], F16)
            # w1b[j] = 2*e[j+1]*rp[j+2]
            nc.vector.tensor_scalar(out=w1a, in0=s_t[:, 1:SW - 1], scalar1=3.0,
                                    scalar2=2.0, op0=OP.is_equal, op1=OP.mult)
            nc.vector.tensor_mul(w1b, w1a, rp_t[:, 2:SW])
            # w5b[j] = 1.5*rp[j+1] + w1b[j]
            nc.vector.tensor_scalar(out=w5a, in0=s_t[:, 1:SW - 1], scalar1=2.0,
                                    scalar2=1.5, op0=OP.is_equal, op1=OP.mult)
            nc.vector.tensor_add(w5b, w5a, w1b)
            # q[j] = e[j+1]*w5b[j+1] + rp[j+1]
            nc.vector.tensor_mul(qa, e_t[:, 1:SH + 1], w5b[:, 1:SH + 1])
            nc.vector.tensor_add(qb, qa, rp_t[:, 1:SH + 1])
            nc.vector.tensor_mul(pzv, lp_t, qb)

            # ---------------- CE chunks ----------------
            for k, cw in enumerate(CHUNKS):
                x_t = x_ts[k]
                e_x = eb.tile([P, C, 512], F16, tag="e")
                nc.scalar.activation(e_x[:, :, 0:cw], x_t, AF.Exp)
                sp = se_ps[k % 4]
                for c in range(C):
                    nc.tensor.matmul(sp[:, 0:cw], lhsT=ident,
                                     rhs=e_x[:, c, 0:cw],
                                     start=(c == 0), stop=(c == C - 1))
                ksl = slice(off[k], off[k + 1])
                # target logit = class-slot 0 of the rotated layout
                nc.vector.tensor_mul(gm[:, ksl], wt_sb[:, ksl], x_t[:, 0, :])
                nc.scalar.activation(lse[:, ksl], sp[:, 0:cw], AF.Ln)
                nc.vector.tensor_mul(wlm[:, ksl], wt_sb[:, ksl], lse[:, ksl])

            # ---- all column reductions after the se matmuls so the PE
            # queue never stalls mid-stream on DVE/ACT results.
            for w in range(SH // WIN):
                acc_mm(("pz",), red[:, RED_PZ:RED_PZ + WIN],
                       lhsT=ones, rhs=pzv[:, w * WIN:(w + 1) * WIN],
                       last=(w == SH // WIN - 1))
            for w in range(4):
                wsl = slice(w * WIN, (w + 1) * WIN)
                acc_mm(("g",), red[:, RED_G:RED_G + WIN], lhsT=ones,
                       rhs=gm[:, wsl], last=(w == 3))
                acc_mm(("wl",), red[:, RED_WL:RED_WL + WIN], lhsT=ones,
                       rhs=wlm[:, wsl], last=(w == 3))
            acc_mm(("gt",), red[:, RED_GT:RED_GT + 48], lhsT=ones,
                   rhs=gm[:, 2048:F], last=True)
            acc_mm(("wlt",), red[:, RED_WLT:RED_WLT + 48], lhsT=ones,
                   rhs=wlm[:, 2048:F], last=True)

            red_sb = stat.tile([1, RED_N], F32)
            nc.vector.tensor_copy(out=red_sb[:, 0:1024], in_=red[:, 0:1024])
            nc.scalar.activation(red_sb[:, 1024:RED_N], red[:, 1024:RED_N],
                                 AF.Copy)
            nc.sync.dma_start(out=red_d, in_=red_sb)

    if compile:
        nc.compile()
    return nc


_program = None


def _get_program():
    global _program
    if _program is None:
        _program = build_program()
    return _program


def _pair_boundary(s):
    """The only clamped boundary pair term not covered on device:
    4 * [s[S-3]==1][s[S-2]==3][s[S-1]==2] per row."""
    m = (s[:, -3] == 1) & (s[:, -2] == 3) & (s[:, -1] == 2)
    return 4.0 * float(m.sum())


def combine_partials(results, s_full, nnz):
    gs = 0.0
    wl = 0.0
    pz = 0.0
    pen = 0.0
    for r in results:
        red = r["red"].astype(np.float64).ravel()
        gs += red[RED_G:RED_G + WIN].sum() + red[RED_GT:RED_GT + 48].sum()
        wl += red[RED_WL:RED_WL + WIN].sum() + red[RED_WLT:RED_WLT + 48].sum()
        pz += red[RED_PZ:RED_PZ + WIN].sum()
        sc = r["pscan"].astype(np.float64)
        pf, mp = -sc[:, 0], -sc[:, 1]   # undo the negated scan
        pfa, mpa = pf[0:RB], mp[0:RB]
        pfb, mpb = pf[RB:P], mp[RB:P]
        pft = pfa + pfb
        mpt = np.minimum(mpa, pfa + mpb)
        pen += (pft - 2.0 * np.minimum(0.0, mpt)).sum()
    pen += 2.0 * pz
    pen += _pair_boundary(s_full)
    ce_loss = (wl - gs) / (B * S)
    penalty = pen / nnz
    return np.float32(ce_loss + PENALTY_WEIGHT * penalty)


def make_in_maps(logits, targets, predicted_structures, ce_weights):
    import ml_dtypes
    lg = np.asarray(logits).astype(ml_dtypes.float8_e4m3fn)
    t = np.asarray(targets, dtype=np.int64)
    w16 = np.asarray(ce_weights, dtype=np.float16)
    s = np.ascontiguousarray(
        np.asarray(predicted_structures).reshape(B, S), dtype=np.float16)
    ident = np.eye(P, dtype=np.float16)
    in_maps = []
    for core in range(NCORES):
        rows = slice(core * RB, (core + 1) * RB)
        tc = t[rows].ravel()
        cnt = np.bincount(tc, minlength=C)
        assert cnt.max() <= PADLEN, f"class count {cnt.max()} > PADLEN"
        perm = np.argsort(tc, kind="stable")
        xs = lg[rows].reshape(N, C)[perm]
        xp = np.zeros((NPAD, C), ml_dtypes.float8_e4m3fn)
        wtp = np.zeros(NPAD, np.float16)
        pos = 0
        for c in range(C):
            band = xs[pos:pos + cnt[c]]
            # rotate class axis: target class -> slot 0
            xp[c * PADLEN:c * PADLEN + cnt[c]] = np.concatenate(
                [band[:, c:], band[:, :c]], axis=1)
            wtp[c * PADLEN:c * PADLEN + cnt[c]] = w16[c]
            pos += cnt[c]
        # [P, F, C] -> class-blocked per chunk [P, sum_k C*w]
        xp = xp.reshape(P, F, C)
        xcore = np.empty((P, F * C), ml_dtypes.float8_e4m3fn)
        o = 0
        a = 0
        for cw in CHUNKS:
            blk = xp[:, a:a + cw, :].transpose(0, 2, 1)  # [P, C, cw]
            xcore[:, o:o + C * cw] = blk.reshape(P, C * cw)
            o += C * cw
            a += cw
        sc = s[rows]
        s_pack = np.zeros((P, SW), np.float16)
        s_pack[0:RB] = sc[:, 0:SW]
        s_pack[RB:P, 0:SH] = sc[:, SH:S]
        in_maps.append({
            "x": xcore,
            "wt": wtp.reshape(P, F),
            "s": s_pack,
            "ident": ident,
        })
    return in_maps


def kernel(logits, targets, predicted_structures, ce_weights):
    in_maps = make_in_maps(logits, targets, predicted_structures, ce_weights)
    t = np.asarray(targets)
    nnz = float(B * S - int((t == 0).sum()))
    s_full = np.asarray(predicted_structures).reshape(B, S)
    nc = _get_program()
    res = run_bass_kernel_spmd(nc, in_maps, core_ids=list(range(NCORES)))
    return combine_partials(res.results, s_full, nnz)


# revision 27
# speedup vs baseline: 1.2148x; 1.0954x over previous
"""Trainium2 Bass kernel for weighted-CE + structural-penalty loss.

Full inputs -> data-parallel shard over batch across 8 NeuronCores ->
per-core Bass kernel computes small fp32 partial sums -> host combines
in float64.

CE trick: the CE term is a plain sum over positions, so it is invariant
under any position permutation. The host sorts each core's positions by
target class into 8 fixed-size bands (PADLEN each, zero-padded), and
rotates the class axis within each band so the target class lands in
class-slot 0. On device the "gather" of the target logit is then just
the contiguous class-0 slice, and the per-position CE weight wt =
ce_weights[targets] (fp16, host-computed) folds into one elementwise
multiply. Padded positions have x=0, wt=0 and contribute exactly 0.

Device per core (positions F=2096 per partition, chunks [48,512x4]):
 - exp on ACT (fp16), se = sum_c exp via identity-lhsT matmuls
   accumulating in PSUM (PE), lse = Ln(se) on ACT.
 - g = sum wt*x_target, wl = sum wt*lse: one TT multiply each + ones
   matmul column reductions into PSUM.
 - penalty (original order, half-rows on 128 partitions): negated
   cumsum scan (rp-lp), row max (=-min P), pair terms via shifted TT
   products reduced by ones-matmuls; host chains the row halves and
   adds the one clamped boundary term.
"""

import numpy as np

import concourse.bass as bass
import concourse.mybir as mybir
import concourse.tile as tile
from concourse import bacc
from concourse.bass_utils import run_bass_kernel_spmd

B, S, C = 512, 4096, 8
PENALTY_WEIGHT = 0.1
NCORES = 8
RB = B // NCORES          # batch rows per core
N = RB * S                # real positions per core (262144)
P = 128                   # SBUF partitions
F = 2096                  # padded positions per partition
PADLEN = F * P // C       # positions per class band (33536)
NPAD = F * P              # padded positions per core
CHUNKS = [128, 512, 512, 512, 384, 48]  # position chunks (sum = F)
SH = S // 2               # penalty half-row length
HALO = 3
SW = SH + HALO

F32 = mybir.dt.float32
F16 = mybir.dt.float16
BF16 = mybir.dt.bfloat16
F8 = mybir.dt.float8e4
OP = mybir.AluOpType
AF = mybir.ActivationFunctionType

# [1, x] reduction regions inside the PSUM "red" tile (4 banks).
# bank0: g main, bank1: wl main, bank2: pz, bank3: tails
RED_G, RED_WL, RED_PZ, RED_GT, RED_WLT = 0, 512, 1024, 1536, 1584
RED_N = 1632
WIN = 512                 # ones-matmul window width


def _patch_act_tables():
    """Prefer the single table set containing Exp+Ln+Copy so the kernel
    pays one ACT_TABLE_LOAD instead of alternating per chunk."""
    import concourse.hw_specs as hw_specs
    if getattr(hw_specs, "_loss_kernel_tables_patched", False):
        return
    orig = hw_specs.get_activation_tables

    def patched(arch):
        t = orig(arch)
        pref = "natural_log_exp_and_others"
        if pref not in t:
            return t
        return {k: (v if k == pref else set()) for k, v in t.items()}

    hw_specs.get_activation_tables = patched
    bacc.get_activation_tables = patched
    hw_specs._loss_kernel_tables_patched = True


def build_program(compile=True):
    _patch_act_tables()
    nc = bacc.Bacc("TRN2", target_bir_lowering=False, debug=False)

    x_d = nc.dram_tensor("x", [P, F * C], F8, kind="ExternalInput").ap()
    wt_d = nc.dram_tensor("wt", [P, F], F16, kind="ExternalInput").ap()
    si_d = nc.dram_tensor("si", [P, SW + P], F16, kind="ExternalInput").ap()
    red_d = nc.dram_tensor("red", [1, RED_N], F32, kind="ExternalOutput").ap()
    ps_d = nc.dram_tensor("pscan", [P, 2], F32, kind="ExternalOutput").ap()

    nch = len(CHUNKS)
    off = [int(x) for x in np.cumsum([0] + CHUNKS)]

    with tile.TileContext(nc) as tc:
        with (
            tc.tile_pool(name="xb", bufs=1) as xb,
            tc.tile_pool(name="eb", bufs=3) as eb,
            tc.tile_pool(name="stat", bufs=1) as stat,
            tc.tile_pool(name="pen", bufs=1) as pen,
            tc.tile_pool(name="psum", bufs=1, space="PSUM") as psum,
        ):
            si_t = pen.tile([P, SW + P], F16)
            s_t = si_t[:, 0:SW]
            ident = si_t[:, SW:SW + P]
            ones = stat.tile([P, 1], F16)
            nc.vector.memset(ones, 1.0)
            wt_sb = stat.tile([P, F], F16)
            x_ts = [xb.tile([P, C, cw], F8, name=f"xt{k}", tag=f"x{k}")
                    for k, cw in enumerate(CHUNKS)]
            # input DMA doorbells all issued up front, in arrival-need order
            nc.sync.dma_start(out=x_ts[0], in_=x_d[:, 0:CHUNKS[0] * C])
            nc.sync.dma_start(out=x_ts[1], in_=x_d[:, off[1] * C:off[2] * C])
            nc.sync.dma_start(out=si_t, in_=si_d)
            nc.sync.dma_start(out=x_ts[2], in_=x_d[:, off[2] * C:off[3] * C])
            nc.sync.dma_start(out=x_ts[3], in_=x_d[:, off[3] * C:off[4] * C])
            nc.sync.dma_start(out=wt_sb, in_=wt_d)
            for k in range(4, len(CHUNKS)):
                nc.sync.dma_start(
                    out=x_ts[k], in_=x_d[:, off[k] * C:off[k + 1] * C])

            lse = stat.tile([P, F], F16)
            gm = stat.tile([P, F], F16)
            wlm = stat.tile([P, F], F16)
            # PSUM: 4 full banks for se (reused) + 4 for the reductions
            se_ps = [psum.tile([P, 512], F32, name=f"se{k}")
                     for k in range(4)]
            red = psum.tile([1, RED_N], F32, name="red")
            started = set()

            def acc_mm(key, out, lhsT, rhs, last):
                st = key not in started
                started.add(key)
                nc.tensor.matmul(out, lhsT=lhsT, rhs=rhs, start=st, stop=last)

            # ---------------- penalty (starts immediately) ----
            lp_t = pen.tile([P, SH], F16)
            rp_t = pen.tile([P, SW], F16)
            e_t = pen.tile([P, SW - 1], F16)
            nc.vector.tensor_scalar(out=lp_t, in0=s_t[:, 0:SH], scalar1=1.0,
                                    scalar2=None, op0=OP.is_equal)
            nc.vector.tensor_scalar(out=rp_t, in0=s_t, scalar1=2.0,
                                    scalar2=None, op0=OP.is_equal)
            nc.vector.tensor_scalar(out=e_t, in0=s_t[:, 0:SW - 1], scalar1=3.0,
                                    scalar2=None, op0=OP.is_equal)

            # negated cumsum: p_t = running(rp - lp) = -P ; fp16 is exact
            # for integer values up to +-2048.
            p_t = pen.tile([P, SH], F16)
            nc.vector.tensor_tensor_scan(out=p_t, data0=rp_t[:, 0:SH],
                                         data1=lp_t, initial=0.0,
                                         op0=OP.add, op1=OP.subtract)
            pscan = stat.tile([P, 2], F32)
            nc.vector.tensor_copy(out=pscan[:, 0:1], in_=p_t[:, SH - 1:SH])
            nc.vector.tensor_reduce(out=pscan[:, 1:2], in_=p_t,
                                    axis=mybir.AxisListType.X, op=OP.max)
            nc.gpsimd.dma_start(out=ps_d, in_=pscan)

            # pair terms: pzv[j] = lp[j]*q[j],
            # q[j] = rp[j+1] + e[j+1]*(1.5*rp[j+2] + 2*e[j+2]*rp[j+3])
            w1a = pen.tile([P, SW - 2], F16)
            w1b = pen.tile([P, SW - 2], F16)
            w5a = pen.tile([P, SW - 2], F16)
            w5b = pen.tile([P, SW - 2], F16)
            qa = pen.tile([P, SH], F16)
            qb = pen.tile([P, SH], F16)
            pzv = pen.tile([P, SH], F16)
            # w1b[j] = 2*e[j+1]*rp[j+2]
            nc.vector.tensor_scalar(out=w1a, in0=s_t[:, 1:SW - 1], scalar1=3.0,
                                    scalar2=2.0, op0=OP.is_equal, op1=OP.mult)
            nc.vector.tensor_mul(w1b, w1a, rp_t[:, 2:SW])
            # w5b[j] = 1.5*rp[j+1] + w1b[j]
            nc.vector.tensor_scalar(out=w5a, in0=s_t[:, 1:SW - 1], scalar1=2.0,
                                    scalar2=1.5, op0=OP.is_equal, op1=OP.mult)
            nc.vector.tensor_add(w5b, w5a, w1b)
            # q[j] = e[j+1]*w5b[j+1] + rp[j+1]
            nc.vector.tensor_mul(qa, e_t[:, 1:SH + 1], w5b[:, 1:SH + 1])
            nc.vector.tensor_add(qb, qa, rp_t[:, 1:SH + 1])
            nc.vector.tensor_mul(pzv, lp_t, qb)

            # ---------------- CE chunks ----------------
            for k, cw in enumerate(CHUNKS):
                x_t = x_ts[k]
                e_x = eb.tile([P, C, 512], F16, tag="e")
                nc.scalar.activation(e_x[:, :, 0:cw], x_t, AF.Exp)
                sp = se_ps[k % 4]
                for c in range(C):
                    nc.tensor.matmul(sp[:, 0:cw], lhsT=ident,
                                     rhs=e_x[:, c, 0:cw],
                                     start=(c == 0), stop=(c == C - 1))
                ksl = slice(off[k], off[k + 1])
                # target logit = class-slot 0 of the rotated layout
                nc.vector.tensor_mul(gm[:, ksl], wt_sb[:, ksl], x_t[:, 0, :])
                nc.scalar.activation(lse[:, ksl], sp[:, 0:cw], AF.Ln)
                nc.vector.tensor_mul(wlm[:, ksl], wt_sb[:, ksl], lse[:, ksl])

            # ---- all column reductions after the se matmuls so the PE
            # queue never stalls mid-stream on DVE/ACT results.
            for w in range(SH // WIN):
                acc_mm(("pz",), red[:, RED_PZ:RED_PZ + WIN],
                       lhsT=ones, rhs=pzv[:, w * WIN:(w + 1) * WIN],
                       last=(w == SH // WIN - 1))
            for w in range(4):
                wsl = slice(w * WIN, (w + 1) * WIN)
                acc_mm(("g",), red[:, RED_G:RED_G + WIN], lhsT=ones,
                       rhs=gm[:, wsl], last=(w == 3))
                acc_mm(("wl",), red[:, RED_WL:RED_WL + WIN], lhsT=ones,
                       rhs=wlm[:, wsl], last=(w == 3))
            acc_mm(("gt",), red[:, RED_GT:RED_GT + 48], lhsT=ones,
                   rhs=gm[:, 2048:F], last=True)
            acc_mm(("wlt",), red[:, RED_WLT:RED_WLT + 48], lhsT=ones,
                   rhs=wlm[:, 2048:F], last=True)

            red_sb = stat.tile([1, RED_N], F32)
            nc.vector.tensor_copy(out=red_sb[:, 0:1024], in_=red[:, 0:1024])
            nc.scalar.activation(red_sb[:, 1024:RED_N], red[:, 1024:RED_N],
                                 AF.Copy)
            nc.sync.dma_start(out=red_d, in_=red_sb)

    if compile:
        nc.compile()
    return nc


_program = None


def _get_program():
    global _program
    if _program is None:
        _program = build_program()
    return _program


def _pair_boundary(s):
    """The only clamped boundary pair term not covered on device:
    4 * [s[S-3]==1][s[S-2]==3][s[S-1]==2] per row."""
    m = (s[:, -3] == 1) & (s[:, -2] == 3) & (s[:, -1] == 2)
    return 4.0 * float(m.sum())


def combine_partials(results, s_full, nnz):
    gs = 0.0
    wl = 0.0
    pz = 0.0
    pen = 0.0
    for r in results:
        red = r["red"].astype(np.float64).ravel()
        gs += red[RED_G:RED_G + WIN].sum() + red[RED_GT:RED_GT + 48].sum()
        wl += red[RED_WL:RED_WL + WIN].sum() + red[RED_WLT:RED_WLT + 48].sum()
        pz += red[RED_PZ:RED_PZ + WIN].sum()
        sc = r["pscan"].astype(np.float64)
        pf, mp = -sc[:, 0], -sc[:, 1]   # undo the negated scan
        pfa, mpa = pf[0:RB], mp[0:RB]
        pfb, mpb = pf[RB:P], mp[RB:P]
        pft = pfa + pfb
        mpt = np.minimum(mpa, pfa + mpb)
        pen += (pft - 2.0 * np.minimum(0.0, mpt)).sum()
    pen += 2.0 * pz
    pen += _pair_boundary(s_full)
    ce_loss = (wl - gs) / (B * S)
    penalty = pen / nnz
    return np.float32(ce_loss + PENALTY_WEIGHT * penalty)


def make_in_maps(logits, targets, predicted_structures, ce_weights):
    import ml_dtypes
    lg = np.asarray(logits).astype(ml_dtypes.float8_e4m3fn)
    t = np.asarray(targets, dtype=np.int64)
    w16 = np.asarray(ce_weights, dtype=np.float16)
    s = np.ascontiguousarray(
        np.asarray(predicted_structures).reshape(B, S), dtype=np.float16)
    ident = np.eye(P, dtype=np.float16)
    in_maps = []
    for core in range(NCORES):
        rows = slice(core * RB, (core + 1) * RB)
        tc = t[rows].ravel()
        cnt = np.bincount(tc, minlength=C)
        assert cnt.max() <= PADLEN, f"class count {cnt.max()} > PADLEN"
        perm = np.argsort(tc, kind="stable")
        xs = lg[rows].reshape(N, C)[perm]
        xp = np.zeros((NPAD, C), ml_dtypes.float8_e4m3fn)
        wtp = np.zeros(NPAD, np.float16)
        pos = 0
        for c in range(C):
            band = xs[pos:pos + cnt[c]]
            # rotate class axis: target class -> slot 0
            xp[c * PADLEN:c * PADLEN + cnt[c]] = np.concatenate(
                [band[:, c:], band[:, :c]], axis=1)
            wtp[c * PADLEN:c * PADLEN + cnt[c]] = w16[c]
            pos += cnt[c]
        # [P, F, C] -> class-blocked per chunk [P, sum_k C*w]
        xp = xp.reshape(P, F, C)
        xcore = np.empty((P, F * C), ml_dtypes.float8_e4m3fn)
        o = 0
        a = 0
        for cw in CHUNKS:
            blk = xp[:, a:a + cw, :].transpose(0, 2, 1)  # [P, C, cw]
            xcore[:, o:o + C * cw] = blk.reshape(P, C * cw)
            o += C * cw
            a += cw
        sc = s[rows]
        s_pack = np.zeros((P, SW), np.float16)
        s_pack[0:RB] = sc[:, 0:SW]
        s_pack[RB:P, 0:SH] = sc[:, SH:S]
        in_maps.append({
            "x": xcore,
            "wt": wtp.reshape(P, F),
            "si": np.concatenate([s_pack, ident], axis=1),
        })
    return in_maps


def kernel(logits, targets, predicted_structures, ce_weights):
    in_maps = make_in_maps(logits, targets, predicted_structures, ce_weights)
    t = np.asarray(targets)
    nnz = float(B * S - int((t == 0).sum()))
    s_full = np.asarray(predicted_structures).reshape(B, S)
    nc = _get_program()
    res = run_bass_kernel_spmd(nc, in_maps, core_ids=list(range(NCORES)))
    return combine_partials(res.results, s_full, nnz)


# revision 29
# speedup vs baseline: 1.2697x; 1.0452x over previous
"""Trainium2 Bass kernel for weighted-CE + structural-penalty loss.

Full inputs -> data-parallel shard over batch across 8 NeuronCores ->
per-core Bass kernel computes small fp32 partial sums -> host combines
in float64.

CE trick: the CE term is a plain sum over positions, so it is invariant
under any position permutation. The host sorts each core's positions by
target class into 8 fixed-size bands (PADLEN each, zero-padded), and
rotates the class axis within each band so the target class lands in
class-slot 0. On device the "gather" of the target logit is then just
the contiguous class-0 slice, and the per-position CE weight wt =
ce_weights[targets] (fp16, host-computed) folds into one elementwise
multiply. Padded positions have x=0, wt=0 and contribute exactly 0.

Device per core (positions F=2096 per partition, chunks [48,512x4]):
 - exp on ACT (fp16), se = sum_c exp via identity-lhsT matmuls
   accumulating in PSUM (PE), lse = Ln(se) on ACT.
 - g = sum wt*x_target, wl = sum wt*lse: one TT multiply each + ones
   matmul column reductions into PSUM.
 - penalty (original order, half-rows on 128 partitions): negated
   cumsum scan (rp-lp), row max (=-min P), pair terms via shifted TT
   products reduced by ones-matmuls; host chains the row halves and
   adds the one clamped boundary term.
"""

import numpy as np

import concourse.bass as bass
import concourse.mybir as mybir
import concourse.tile as tile
from concourse import bacc
from concourse.bass_utils import run_bass_kernel_spmd

B, S, C = 512, 4096, 8
PENALTY_WEIGHT = 0.1
NCORES = 8
RB = B // NCORES          # batch rows per core
N = RB * S                # real positions per core (262144)
P = 128                   # SBUF partitions
F = 2096                  # padded positions per partition
PADLEN = F * P // C       # positions per class band (33536)
NPAD = F * P              # padded positions per core
CHUNKS = [128, 512, 512, 512, 384, 48]  # position chunks (sum = F)
SH = S // 2               # penalty half-row length
HALO = 3
SW = SH + HALO

F32 = mybir.dt.float32
F16 = mybir.dt.float16
BF16 = mybir.dt.bfloat16
F8 = mybir.dt.float8e4
OP = mybir.AluOpType
AF = mybir.ActivationFunctionType

# [1, x] reduction regions inside the PSUM "red" tile (4 banks).
# bank0: g main, bank1: wl main, bank2: pz, bank3: tails
RED_G, RED_WL, RED_PZ, RED_GT, RED_WLT = 0, 512, 1024, 1536, 1584
RED_N = 1632
WIN = 512                 # ones-matmul window width


def _patch_act_tables():
    """Prefer the single table set containing Exp+Ln+Copy so the kernel
    pays one ACT_TABLE_LOAD instead of alternating per chunk."""
    import concourse.hw_specs as hw_specs
    if getattr(hw_specs, "_loss_kernel_tables_patched", False):
        return
    orig = hw_specs.get_activation_tables

    def patched(arch):
        t = orig(arch)
        pref = "natural_log_exp_and_others"
        if pref not in t:
            return t
        return {k: (v if k == pref else set()) for k, v in t.items()}

    hw_specs.get_activation_tables = patched
    bacc.get_activation_tables = patched
    hw_specs._loss_kernel_tables_patched = True


def build_program(compile=True):
    _patch_act_tables()
    nc = bacc.Bacc("TRN2", target_bir_lowering=False, debug=False)

    x_d = nc.dram_tensor("x", [P, F * C], F8, kind="ExternalInput").ap()
    wt_d = nc.dram_tensor("wt", [P, F], F16, kind="ExternalInput").ap()
    si_d = nc.dram_tensor("si", [P, SW + P + 1], F16, kind="ExternalInput").ap()
    red_d = nc.dram_tensor("red", [1, RED_N], F32, kind="ExternalOutput").ap()
    ps_d = nc.dram_tensor("pscan", [P, 2], F32, kind="ExternalOutput").ap()

    nch = len(CHUNKS)
    off = [int(x) for x in np.cumsum([0] + CHUNKS)]

    with tile.TileContext(nc) as tc:
        with (
            tc.tile_pool(name="xb", bufs=1) as xb,
            tc.tile_pool(name="eb", bufs=3) as eb,
            tc.tile_pool(name="stat", bufs=1) as stat,
            tc.tile_pool(name="pen", bufs=1) as pen,
            tc.tile_pool(name="psum", bufs=1, space="PSUM") as psum,
        ):
            si_t = pen.tile([P, SW + P + 1], F16)
            s_t = si_t[:, 0:SW]
            ident = si_t[:, SW:SW + P]
            wscale16 = si_t[:, SW + P:SW + P + 1]
            wscale = stat.tile([P, 1], F32)
            ones = stat.tile([P, 1], F16)
            nc.vector.memset(ones, 1.0)
            wt_sb = stat.tile([P, F], F16)
            x_ts = [xb.tile([P, C, cw], F8, name=f"xt{k}", tag=f"x{k}")
                    for k, cw in enumerate(CHUNKS)]
            # input DMA doorbells all issued up front, in arrival-need order
            nc.sync.dma_start(out=si_t, in_=si_d)
            nc.sync.dma_start(out=x_ts[0], in_=x_d[:, 0:CHUNKS[0] * C])
            nc.sync.dma_start(out=x_ts[1], in_=x_d[:, off[1] * C:off[2] * C])
            nc.sync.dma_start(out=x_ts[2], in_=x_d[:, off[2] * C:off[3] * C])
            nc.sync.dma_start(out=x_ts[3], in_=x_d[:, off[3] * C:off[4] * C])
            nc.sync.dma_start(out=wt_sb, in_=wt_d)
            for k in range(4, len(CHUNKS)):
                nc.sync.dma_start(
                    out=x_ts[k], in_=x_d[:, off[k] * C:off[k + 1] * C])

            lse = stat.tile([P, F], F16)
            gm = stat.tile([P, F], F16)
            wlm = stat.tile([P, F], F16)
            # PSUM: 4 full banks for se (reused) + 4 for the reductions
            se_ps = [psum.tile([P, 512], F32, name=f"se{k}")
                     for k in range(4)]
            red = psum.tile([1, RED_N], F32, name="red")
            started = set()

            def acc_mm(key, out, lhsT, rhs, last):
                st = key not in started
                started.add(key)
                nc.tensor.matmul(out, lhsT=lhsT, rhs=rhs, start=st, stop=last)

            nc.vector.tensor_copy(out=wscale, in_=wscale16)

            # ---------------- penalty (starts immediately) ----
            lp_t = pen.tile([P, SH], F16)
            rp_t = pen.tile([P, SW], F16)
            e_t = pen.tile([P, SW - 1], F16)
            nc.vector.tensor_scalar(out=lp_t, in0=s_t[:, 0:SH], scalar1=1.0,
                                    scalar2=None, op0=OP.is_equal)
            nc.vector.tensor_scalar(out=rp_t, in0=s_t, scalar1=2.0,
                                    scalar2=None, op0=OP.is_equal)
            nc.vector.tensor_scalar(out=e_t, in0=s_t[:, 0:SW - 1], scalar1=3.0,
                                    scalar2=None, op0=OP.is_equal)

            # negated cumsum: p_t = running(rp - lp) = -P ; fp16 is exact
            # for integer values up to +-2048.
            p_t = pen.tile([P, SH], F16)
            nc.vector.tensor_tensor_scan(out=p_t, data0=rp_t[:, 0:SH],
                                         data1=lp_t, initial=0.0,
                                         op0=OP.add, op1=OP.subtract)
            pscan = stat.tile([P, 2], F32)
            nc.vector.tensor_copy(out=pscan[:, 0:1], in_=p_t[:, SH - 1:SH])
            nc.vector.pool(out=pscan[:, 1:2],
                           in_=p_t.rearrange("p (a b) -> p a b", a=1),
                           func=mybir.PoolFunctionType.max)
            nc.gpsimd.dma_start(out=ps_d, in_=pscan)

            # pair terms: pzv[j] = lp[j]*q[j],
            # q[j] = rp[j+1] + e[j+1]*(1.5*rp[j+2] + 2*e[j+2]*rp[j+3])
            w1a = pen.tile([P, SW - 2], F16)
            w1b = pen.tile([P, SW - 2], F16)
            w5a = pen.tile([P, SW - 2], F16)
            w5b = pen.tile([P, SW - 2], F16)
            qa = pen.tile([P, SH], F16)
            qb = pen.tile([P, SH], F16)
            pzv = pen.tile([P, SH], F16)
            # w1b[j] = 2*e[j+1]*rp[j+2]
            nc.vector.tensor_scalar(out=w1a, in0=s_t[:, 1:SW - 1], scalar1=3.0,
                                    scalar2=2.0, op0=OP.is_equal, op1=OP.mult)
            nc.vector.tensor_mul(w1b, w1a, rp_t[:, 2:SW])
            # w5b[j] = 1.5*rp[j+1] + w1b[j]
            nc.vector.tensor_scalar(out=w5a, in0=s_t[:, 1:SW - 1], scalar1=2.0,
                                    scalar2=1.5, op0=OP.is_equal, op1=OP.mult)
            nc.vector.tensor_add(w5b, w5a, w1b)
            # q[j] = e[j+1]*w5b[j+1] + rp[j+1]
            nc.vector.tensor_mul(qa, e_t[:, 1:SH + 1], w5b[:, 1:SH + 1])
            nc.vector.tensor_add(qb, qa, rp_t[:, 1:SH + 1])
            nc.vector.tensor_mul(pzv, lp_t, qb)

            # ---------------- CE chunks ----------------
            for k, cw in enumerate(CHUNKS):
                x_t = x_ts[k]
                e_x = eb.tile([P, C, 512], F16, tag="e")
                nc.scalar.activation(e_x[:, :, 0:cw], x_t, AF.Exp)
                sp = se_ps[k % 4]
                for c in range(C):
                    nc.tensor.matmul(sp[:, 0:cw], lhsT=ident,
                                     rhs=e_x[:, c, 0:cw],
                                     start=(c == 0), stop=(c == C - 1))
                ksl = slice(off[k], off[k + 1])
                # target logit = class-slot 0 of the rotated layout;
                # wt is constant per 16-partition class band and pads have
                # x=0, so gm = w_band * x0 on the (gappier) ACT engine.
                nc.scalar.activation(gm[:, ksl], x_t[:, 0, :], AF.Copy,
                                     scale=wscale)
                nc.scalar.activation(lse[:, ksl], sp[:, 0:cw], AF.Ln)
                nc.vector.tensor_mul(wlm[:, ksl], wt_sb[:, ksl], lse[:, ksl])

            # ---- all column reductions after the se matmuls so the PE
            # queue never stalls mid-stream on DVE/ACT results.
            for w in range(SH // WIN):
                acc_mm(("pz",), red[:, RED_PZ:RED_PZ + WIN],
                       lhsT=ones, rhs=pzv[:, w * WIN:(w + 1) * WIN],
                       last=(w == SH // WIN - 1))
            for w in range(4):
                wsl = slice(w * WIN, (w + 1) * WIN)
                acc_mm(("g",), red[:, RED_G:RED_G + WIN], lhsT=ones,
                       rhs=gm[:, wsl], last=(w == 3))
                acc_mm(("wl",), red[:, RED_WL:RED_WL + WIN], lhsT=ones,
                       rhs=wlm[:, wsl], last=(w == 3))
            acc_mm(("gt",), red[:, RED_GT:RED_GT + 48], lhsT=ones,
                   rhs=gm[:, 2048:F], last=True)
            acc_mm(("wlt",), red[:, RED_WLT:RED_WLT + 48], lhsT=ones,
                   rhs=wlm[:, 2048:F], last=True)

            red_sb = stat.tile([1, RED_N], F32)
            nc.vector.tensor_copy(out=red_sb[:, 0:1024], in_=red[:, 0:1024])
            nc.scalar.activation(red_sb[:, 1024:RED_N], red[:, 1024:RED_N],
                                 AF.Copy)
            nc.sync.dma_start(out=red_d, in_=red_sb)

    if compile:
        nc.compile()
    return nc


_program = None


def _get_program():
    global _program
    if _program is None:
        _program = build_program()
    return _program


def _pair_boundary(s):
    """The only clamped boundary pair term not covered on device:
    4 * [s[S-3]==1][s[S-2]==3][s[S-1]==2] per row."""
    m = (s[:, -3] == 1) & (s[:, -2] == 3) & (s[:, -1] == 2)
    return 4.0 * float(m.sum())


def combine_partials(results, s_full, nnz):
    gs = 0.0
    wl = 0.0
    pz = 0.0
    pen = 0.0
    for r in results:
        red = r["red"].astype(np.float64).ravel()
        gs += red[RED_G:RED_G + WIN].sum() + red[RED_GT:RED_GT + 48].sum()
        wl += red[RED_WL:RED_WL + WIN].sum() + red[RED_WLT:RED_WLT + 48].sum()
        pz += red[RED_PZ:RED_PZ + WIN].sum()
        sc = r["pscan"].astype(np.float64)
        pf, mp = -sc[:, 0], -sc[:, 1]   # undo the negated scan
        pfa, mpa = pf[0:RB], mp[0:RB]
        pfb, mpb = pf[RB:P], mp[RB:P]
        pft = pfa + pfb
        mpt = np.minimum(mpa, pfa + mpb)
        pen += (pft - 2.0 * np.minimum(0.0, mpt)).sum()
    pen += 2.0 * pz
    pen += _pair_boundary(s_full)
    ce_loss = (wl - gs) / (B * S)
    penalty = pen / nnz
    return np.float32(ce_loss + PENALTY_WEIGHT * penalty)


def make_in_maps(logits, targets, predicted_structures, ce_weights):
    import ml_dtypes
    lg = np.asarray(logits).astype(ml_dtypes.float8_e4m3fn)
    t = np.asarray(targets, dtype=np.int64)
    w16 = np.asarray(ce_weights, dtype=np.float16)
    s = np.ascontiguousarray(
        np.asarray(predicted_structures).reshape(B, S), dtype=np.float16)
    ident = np.eye(P, dtype=np.float16)
    in_maps = []
    for core in range(NCORES):
        rows = slice(core * RB, (core + 1) * RB)
        tc = t[rows].ravel()
        cnt = np.bincount(tc, minlength=C)
        assert cnt.max() <= PADLEN, f"class count {cnt.max()} > PADLEN"
        perm = np.argsort(tc, kind="stable")
        xs = lg[rows].reshape(N, C)[perm]
        xp = np.zeros((NPAD, C), ml_dtypes.float8_e4m3fn)
        wtp = np.zeros(NPAD, np.float16)
        pos = 0
        for c in range(C):
            band = xs[pos:pos + cnt[c]]
            # rotate class axis: target class -> slot 0
            xp[c * PADLEN:c * PADLEN + cnt[c]] = np.concatenate(
                [band[:, c:], band[:, :c]], axis=1)
            wtp[c * PADLEN:c * PADLEN + cnt[c]] = w16[c]
            pos += cnt[c]
        # [P, F, C] -> class-blocked per chunk [P, sum_k C*w]
        xp = xp.reshape(P, F, C)
        xcore = np.empty((P, F * C), ml_dtypes.float8_e4m3fn)
        o = 0
        a = 0
        for cw in CHUNKS:
            blk = xp[:, a:a + cw, :].transpose(0, 2, 1)  # [P, C, cw]
            xcore[:, o:o + C * cw] = blk.reshape(P, C * cw)
            o += C * cw
            a += cw
        wcol = np.repeat(w16, P // C).reshape(P, 1)
        sc = s[rows]
        s_pack = np.zeros((P, SW), np.float16)
        s_pack[0:RB] = sc[:, 0:SW]
        s_pack[RB:P, 0:SH] = sc[:, SH:S]
        in_maps.append({
            "x": xcore,
            "wt": wtp.reshape(P, F),
            "si": np.concatenate([s_pack, ident, wcol], axis=1),
        })
    return in_maps


def kernel(logits, targets, predicted_structures, ce_weights):
    in_maps = make_in_maps(logits, targets, predicted_structures, ce_weights)
    t = np.asarray(targets)
    nnz = float(B * S - int((t == 0).sum()))
    s_full = np.asarray(predicted_structures).reshape(B, S)
    nc = _get_program()
    res = run_bass_kernel_spmd(nc, in_maps, core_ids=list(range(NCORES)))
    return combine_partials(res.results, s_full, nnz)
